# revision 2
# baseline (speedup 1.0000x reference)
"""Trainium2 Bass kernel for nn_BinaryMemory (retrieval_knn).

reference:
    gated = sigmoid(query @ W.T + b)                      # [1, D], D=4096
    sims  = 1 - mean(|memory - gated|, axis=-1)           # [N],   N=16384
    mask  = sims >= 0.8

Sharding (8 cores, no collectives): shard the D axis; core c owns
d-chunk [c*512, (c+1)*512). All bulk tensors stream as fp8_e3m4.
Layout is d-on-partitions (memory shard transposed host-side to
[512 d, 16384 n]) so the gate value g[d] is a per-partition scalar.

|m-g| split: DVE pieces compute min(m-g,0) via one fused
tensor_scalar(sub, min); the m-term sum rides on the PE (ones^T @ m,
gate-independent, issued at DMA-land) and the g-term is corrected on
host per (k, group). ScalarE pieces compute |m-g| in one op via
activation(Abs, scale=-1, bias=+g). Reductions over d run on the PE
into psum rows at quadrant offsets (4-way tile_position concurrency).

v2 vs baseline (55.1us): trace showed (a) the first gate matmul
stalled to 11.5us on the constants DMA completion (issued after W on
the ring + ~2us HBM receipt latency) -> c8 now rides FIRST on the sync
ring; (b) a 4.4us serial gate tail -> replaced by zcp fp16 copy (DVE,
psum 2x), 4 fused strip-sum+transpose matmuls (stationary = zcp
chunks, moving = selsum column), one [128,4] add (+b) and one Sigmoid;
the negate op is gone (Abs uses scale=-1, bias=+g); (c) the gate-sum
correction chain ran last, tailing the kernel -> now issued right
after the gate; (d) the last two memory tiles are split into half-DMAs
with per-half elementwise + reduction so the kernel tail after the
final DMA completion (~receipt-latency-bound) is one [128,2048] DVE op
instead of a full tile; (e) outputs go out per-k on the scalar HWDGE
ring (SWDGE descriptor rings off the critical path).
"""
import sys

sys.path.insert(0, "/opt/trn_rl_repo")

import numpy as np
import ml_dtypes

import concourse.bacc as bacc
import concourse.mybir as mybir
import concourse.tile as tile
from concourse.bass_utils import run_bass_kernel_spmd

N_CORES = 8
D = 4096
N = 16384
D_SH = D // N_CORES            # 512 dims per core
DC = D_SH // 128               # 4 d-chunks (partition blocks)
NT = 4096                      # n per tile
NK = N // NT                   # 4 n-chunks
NG = NT // 512                 # 8 psum groups per tile
THRESHOLD = 0.8

# pieces: (k, c) -> list of (n_lo, n_hi, engine) covering [0, NT)
# engine: 'A' = ScalarE Abs path, 'V' = DVE sub+min path
_FULL_A = {(0, 1), (1, 0), (1, 2), (2, 0), (2, 2), (3, 0)}


def _pieces(k, c):
    if (k, c) == (3, 2):
        return [(0, 2048, "A"), (2048, 4096, "V")]
    if (k, c) == (3, 3):
        return [(0, 2048, "V"), (2048, 4096, "V")]
    return [(0, 4096, "A" if (k, c) in _FULL_A else "V")]


# host correction: for each (k, group j), chunks whose covering piece is DVE
def _dve_chunks(k, j):
    out = []
    for c in range(DC):
        for lo, hi, eng in _pieces(k, c):
            if lo <= j * 512 < hi and eng == "V":
                out.append(c)
    return out


_CACHE = {}


def _build():
    f32 = mybir.dt.float32
    f16 = mybir.dt.float16
    f8 = mybir.dt.float8e3
    A = mybir.AluOpType
    AF = mybir.ActivationFunctionType
    nc = bacc.Bacc(
        "TRN2", target_bir_lowering=False, debug=False, num_devices=N_CORES
    )

    memT = nc.dram_tensor("memT", [D_SH, N], f8, kind="ExternalInput")
    # W shard, host-packed: partition p, chunk j holds W.T[j*128 + p, :]
    wtp = nc.dram_tensor("wtp", [128, 32 * D_SH], f8, kind="ExternalInput")
    # packed constants: cols 0:32 qcol, 32 ones, 33 neg2, 34 selsum
    c8 = nc.dram_tensor("c8", [128, 35], f8, kind="ExternalInput")
    # packed f32 constants: cols 0:4 b columns, 4 ones
    c32 = nc.dram_tensor("c32", [128, 5], f32, kind="ExternalInput")
    outp = nc.dram_tensor("outp", [33, 512], f32, kind="ExternalOutput")

    with tile.TileContext(nc) as tc:
        with (
            tc.tile_pool(name="wts", bufs=1) as wpool,
            tc.tile_pool(name="mem", bufs=9) as mpool,
            tc.tile_pool(name="memh", bufs=4) as mhpool,
            tc.tile_pool(name="dts", bufs=3) as dpool,
            tc.tile_pool(name="acts", bufs=2) as apool,
            tc.tile_pool(name="cp", bufs=8) as cppool,
            tc.tile_pool(name="small", bufs=1) as spool,
            tc.tile_pool(name="psg", bufs=1, space="PSUM") as ppg,
            tc.tile_pool(name="psm", bufs=7, space="PSUM") as ppm,
        ):
            # constants FIRST on the sync ring: everything upstream of the
            # gate stalls on the first DMA completion (~2us HBM receipt
            # after ~1.5us SDMA spin-up), so the q columns must be in it.
            c8_sb = spool.tile([128, 35], f8, tag="c8")
            nc.sync.dma_start(out=c8_sb[:], in_=c8[:])
            wts = []
            for h in range(8):
                wt_sb = wpool.tile([128, 4 * D_SH], f8, tag=f"wt{h}")
                nc.sync.dma_start(
                    out=wt_sb[:],
                    in_=wtp[:, h * 4 * D_SH : (h + 1) * 4 * D_SH],
                )
                wts.append(wt_sb)
            c32_sb = spool.tile([128, 5], f32, tag="c32")
            nc.scalar.dma_start(out=c32_sb[:], in_=c32[:])
            qc_sb = c8_sb[:, 0:32]
            ones_sb = c8_sb[:, 32:33]
            neg2_sb = c8_sb[:, 33:34]
            selsum8 = c8_sb[:, 34:35]
            b4 = c32_sb[:, 0:4]
            ones32_sb = c32_sb[:, 4:5]
            # preload Sigmoid+Abs activation tables off the critical path
            # (no DMA dependency: dummy input is memset on-device)
            dum_in = spool.tile([1, 4], f32, tag="dumin")
            nc.vector.memset(dum_in[:], 0.25)
            dum = spool.tile([1, 4], f32, tag="dum")
            nc.scalar.activation(dum[:], dum_in[:], AF.Sigmoid)
            nc.scalar.activation(dum[:], dum_in[:], AF.Abs)

            # ---- gate: 4 quadrant strips accumulate partial z rows ----
            zps = ppg.tile([128, D_SH], f32, tag="z")
            # zero the bank: zcp reads all 128 partitions and stale psum
            # bits can be NaN (NaN*0 = NaN would poison the strip-sum)
            nc.vector.memset(zps[:], 0.0)
            for j in range(32):
                r = j % 4
                nc.tensor.matmul(
                    zps[32 * r : 32 * r + 1, :],
                    qc_sb[:, j : j + 1],
                    wts[j // 4][:, (j % 4) * D_SH : (j % 4 + 1) * D_SH],
                    start=(j < 4),
                    stop=(j >= 28),
                    tile_position=(0, 32 * r),
                    skip_group_check=True,
                )
            hp = tc.high_priority()
            hp.__enter__()
            # psum -> SBUF fp16 on the DVE (2x mode, ~0.4us)
            zcp = spool.tile([128, D_SH], f16, tag="zcp")
            nc.vector.tensor_copy(zcp[:], zps[:])
            # fused strip-sum + transpose: stationary = zcp 128-col chunk,
            # moving = selsum column (1 at partitions {0,32,64,96}) ->
            # psum col c holds z[c*128 + p] per partition p
            ztp = zps[:, 504:508]
            for c in range(DC):
                nc.tensor.matmul(
                    ztp[:, c : c + 1],
                    zcp[:, c * 128 : (c + 1) * 128],
                    selsum8,
                    start=True,
                    stop=True,
                    skip_group_check=True,
                )
            zb = spool.tile([128, DC], f32, tag="zb")
            nc.vector.tensor_tensor(zb[:], ztp, b4, A.add)
            gpos = spool.tile([128, DC], f32, tag="gpos")
            nc.scalar.activation(gpos[:], zb[:], AF.Sigmoid)
            # per-d-chunk gate sums for the host-side m-term correction;
            # issued here (early) so nothing trails the kernel end
            gs = zps[0:1, 500:504]
            nc.tensor.matmul(
                gs, ones32_sb, gpos[:], start=True, stop=True,
                skip_group_check=True,
            )
            gs_sb = spool.tile([1, DC], f32, tag="gs")
            nc.scalar.activation(gs_sb[:], gs, AF.Copy)
            nc.scalar.dma_start(out=outp[32:33, 0:DC], in_=gs_sb[:])
            hp.__exit__(None, None, None)

            # ---- main loop ----
            # phase 1 issues the gate-independent m-term matmuls right at
            # each piece's DMA; phase 2 runs elementwise + reduction.
            for k in range(NK):
                bank0 = ppm.tile([128, 512], f32, tag="bank")
                bank1 = ppm.tile([128, 512], f32, tag="bank")
                banks = [bank0, bank1]
                # per-group pass counts for psum start/stop bookkeeping
                total = [0] * NG
                for c in range(DC):
                    for lo, hi, eng in _pieces(k, c):
                        npass = 1 if eng == "A" else 2
                        for j in range(lo // 512, hi // 512):
                            total[j] += npass
                seen = [0] * NG
                pieces = []
                for c in range(DC):
                    for lo, hi, eng in _pieces(k, c):
                        w = hi - lo
                        pool = mpool if w == NT else mhpool
                        mt = pool.tile(
                            [128, w], f8, tag="m" if w == NT else "mh"
                        )
                        nc.sync.dma_start(
                            out=mt[:],
                            in_=memT[
                                c * 128 : (c + 1) * 128,
                                k * NT + lo : k * NT + hi,
                            ],
                        )
                        pieces.append((c, lo, hi, eng, mt))
                        if eng == "V":
                            for j in range(lo // 512, hi // 512):
                                s = slice((j * 512) - lo, (j * 512) - lo + 512)
                                nc.tensor.matmul(
                                    banks[j // 4][32 * (j % 4) : 32 * (j % 4) + 1, :],
                                    ones_sb,
                                    mt[:, s],
                                    start=(seen[j] == 0),
                                    stop=(seen[j] == total[j] - 1),
                                    tile_position=(0, 32 * (j % 4)),
                                    skip_group_check=True,
                                )
                                seen[j] += 1
                for c, lo, hi, eng, mt in pieces:
                    w = hi - lo
                    if eng == "A":
                        at = apool.tile([128, w], f8, tag="a" if w == NT else "ah")
                        nc.scalar.activation(
                            at[:], mt[:], AF.Abs,
                            bias=gpos[:, c : c + 1], scale=-1.0,
                        )
                        src_, stat = at, ones_sb
                    else:
                        dt = dpool.tile([128, w], f8, tag="d" if w == NT else "dh")
                        nc.vector.tensor_scalar(
                            dt[:], mt[:],
                            gpos[:, c : c + 1], 0.0,
                            A.subtract, A.min,
                        )
                        src_, stat = dt, neg2_sb
                    for j in range(lo // 512, hi // 512):
                        s = slice((j * 512) - lo, (j * 512) - lo + 512)
                        nc.tensor.matmul(
                            banks[j // 4][32 * (j % 4) : 32 * (j % 4) + 1, :],
                            stat,
                            src_[:, s],
                            start=(seen[j] == 0),
                            stop=(seen[j] == total[j] - 1),
                            tile_position=(0, 32 * (j % 4)),
                            skip_group_check=True,
                        )
                        seen[j] += 1
                for h in range(2):
                    cp = cppool.tile([128, 512], f32, tag="cp")
                    if h == 0:
                        nc.scalar.activation(cp[:], banks[h][:], AF.Copy)
                    else:
                        nc.vector.tensor_copy(cp[:], banks[h][:])
                    nc.scalar.dma_start(
                        out=outp[8 * k + 4 * h : 8 * k + 4 * h + 4, :],
                        in_=cp[0:128:32, :],
                    )

    nc.compile()
    return nc


def _get_nc():
    if "nc" not in _CACHE:
        _CACHE["nc"] = _build()
    return _CACHE["nc"]


def kernel(query, W, b, memory, _trace=False, _return_raw=False):
    f8 = ml_dtypes.float8_e3m4
    query = np.asarray(query, dtype=np.float32)
    W = np.asarray(W, dtype=np.float32)
    b = np.asarray(b, dtype=np.float32)
    memory = np.asarray(memory, dtype=np.float32)

    mem8T = np.ascontiguousarray(memory.astype(f8).T)       # [D, N] fp8
    W8 = W.astype(f8)
    q8 = query.reshape(32, 128).astype(f8).T                # [128, 32]
    c8 = np.zeros((128, 35), dtype=f8)
    c8[:, 0:32] = q8
    c8[:, 32] = f8(1.0)
    c8[:, 33] = f8(-2.0)
    c8[0:128:32, 34] = f8(1.0)

    in_maps = []
    for c in range(N_CORES):
        sl = slice(c * D_SH, (c + 1) * D_SH)
        # wtp[p, j*512 + n] = W.T[j*128 + p, n] = W8[sl][n, j*128+p]
        wsh = W8[sl, :]                       # [512, 4096]
        wtp = np.ascontiguousarray(
            wsh.T.reshape(32, 128, D_SH).transpose(1, 0, 2).reshape(128, -1)
        )
        c32 = np.zeros((128, 5), dtype=np.float32)
        c32[:, 0:4] = b[sl].reshape(4, 128).T
        c32[:, 4] = 1.0
        in_maps.append(
            {
                "memT": np.ascontiguousarray(mem8T[sl, :]),
                "wtp": wtp,
                "c8": c8,
                "c32": c32,
            }
        )

    nc = _get_nc()
    res = run_bass_kernel_spmd(
        nc, in_maps, list(range(N_CORES)), trace=_trace
    )

    total = np.zeros(N, dtype=np.float64)
    for c in range(N_CORES):
        out = res.results[c]["outp"]
        gsum = out[32, 0:DC].astype(np.float64)   # sum of g per d-chunk
        rows = out[0:32].reshape(NK, NG, 512)
        corr = np.array(
            [
                [sum(gsum[ci] for ci in _dve_chunks(k, j)) for j in range(NG)]
                for k in range(NK)
            ]
        )
        total += (rows - corr[:, :, None]).reshape(N)
    sims = (1.0 - total / D).astype(np.float32)
    mask = sims >= THRESHOLD
    if _return_raw:
        return (sims, mask), res
    return sims, mask
